# revision 1
# baseline (speedup 1.0000x reference)
"""AugGraphConv (per-relation GAT + lang-level softmax) on 8 TRN2 NeuronCores.

Strategy (dst-sharded graph parallel):
  - Nodes padded to NPAD=50176; core m owns rows [m*6272, (m+1)*6272).
  - Per-core x is host-permuted so owned rows are tiles 0..48 (SPMD program).
  - Stage A (per core, replicated over all nodes): LayerNorm, then per-relation
    feat_r = xn @ [W_r | u_r]  (u_r folds att_src so al = feat[:,128:136]),
    plus ar (att_dst logits) and self path for owned rows.
  - Stage B: edges binned by (own dst tile, relation), chunks of 128 edges.
    Indirect-DMA gather of feat rows by src; one-hot selection matrix S built
    with is_equal vs iota; segment softmax without max-subtraction (logits are
    O(1) bounded): w=exp(leaky(al_src+ar_dst)); num/den accumulate in PSUM via
    S^T matmuls. Padded edges get dst_local=200 -> zero S column -> dropped.
  - Lang stage fused per tile: softmax over 6 feature rows, gelu, residual.
"""

import os
import numpy as np
import ml_dtypes
from contextlib import ExitStack

import concourse.bass as bass
import concourse.mybir as mybir
from concourse.bass import IndirectOffsetOnAxis
from concourse.tile import TileContext
from concourse.bass_utils import run_bass_kernel_spmd

N, D, H, R, C = 50000, 128, 8, 5, 16
P = 128
M = 8
NPAD = 50176            # 392 * 128, divisible by M*P
S = NPAD // M           # 6272 rows per core
T = S // P              # 49 owned tiles per core
GT = NPAD // P          # 392 global tiles
FD = D + H              # 136: [xw | al]
F32 = mybir.dt.float32
BF16 = mybir.dt.bfloat16
I32 = mybir.dt.int32
AF = mybir.ActivationFunctionType
ALU = mybir.AluOpType
AX = mybir.AxisListType
NEGM = -30.0            # softmax mask value (exp(-30) ~ 1e-13, negligible)

LAST_RESULTS = None     # test.py reads exec_time_ns / profile from here


def _split_multiwaits(nc):
    """This toolchain's walrus codegen allows only one sem-wait per
    instruction; hoist extra waits into preceding NoOps on the same engine
    (sequencer executes them in program order, so semantics are identical)."""
    n_split = 0
    for _, bbwrap in nc.bb_map.items():
        bb = bbwrap.bb
        out = []
        changed = False
        for inst in list(bb.instructions):
            si = inst.sync_info
            if si is not None and si.on_wait is not None and len(si.on_wait) > 1:
                waits = list(si.on_wait)
                for w in waits[:-1]:
                    out.append(mybir.InstNoOp(
                        name=nc.get_next_instruction_name(),
                        engine=inst.engine, ins=[], outs=[],
                        sync_info=mybir.SyncInfo(on_wait=[w], on_update=[])))
                    n_split += 1
                si.on_wait = waits[-1:]
                inst.sync_info = si
                changed = True
            out.append(inst)
        if changed:
            bb.instructions = out
    return n_split


def _build(K, TOTC):
    nc = bass.Bass()
    x_full = nc.declare_dram_parameter("x_full", [NPAD, D], F32, isOutput=False)
    srcg = nc.declare_dram_parameter("src_gidx", [P, TOTC], I32, isOutput=False)
    argi = nc.declare_dram_parameter("ar_gidx", [P, TOTC], I32, isOutput=False)
    dstl = nc.declare_dram_parameter("dstl_f", [P, TOTC], BF16, isOutput=False)
    wcat = nc.declare_dram_parameter("wcat", [D, R * FD], BF16, isOutput=False)
    vcat = nc.declare_dram_parameter("vcat", [D, R * H], BF16, isOutput=False)
    wself = nc.declare_dram_parameter("wself", [D, D], BF16, isOutput=False)
    wcross = nc.declare_dram_parameter("wcross", [D, D], F32, isOutput=False)
    asl = nc.declare_dram_parameter("asl_rep", [P, D], F32, isOutput=False)
    adl = nc.declare_dram_parameter("adl_rep", [P, D], F32, isOutput=False)
    bw = nc.declare_dram_parameter("bw_rep", [P, R * D], F32, isOutput=False)
    bl = nc.declare_dram_parameter("bl_rep", [P, D], F32, isOutput=False)
    iota = nc.declare_dram_parameter("iota_f", [P, P], BF16, isOutput=False)
    iden = nc.declare_dram_parameter("ident_f", [P, P], F32, isOutput=False)
    out = nc.declare_dram_parameter("out", [S, D], F32, isOutput=True)

    feat = nc.dram_tensor("feat_all", [R * NPAD, FD], BF16)
    arrel = nc.dram_tensor("ar_rel", [R * S, H], BF16)
    sown = nc.dram_tensor("self_own", [S, D], F32)

    with TileContext(nc) as tc, ExitStack() as ctx:
        cp = ctx.enter_context(tc.tile_pool(name="const", bufs=1))
        sb = ctx.enter_context(tc.tile_pool(name="sb", bufs=3))
        eb = ctx.enter_context(tc.tile_pool(name="eb", bufs=4))
        lb = ctx.enter_context(tc.tile_pool(name="lb", bufs=2))
        psA = ctx.enter_context(tc.tile_pool(name="psA", bufs=2, space="PSUM"))
        psB = ctx.enter_context(tc.tile_pool(name="psB", bufs=1, space="PSUM"))

        # ---- persistent constants / index arrays ----
        wcat_s = cp.tile([D, R * FD], BF16)
        nc.gpsimd.dma_start(out=wcat_s[:], in_=wcat[:])
        vcat_s = cp.tile([D, R * H], BF16)
        nc.gpsimd.dma_start(out=vcat_s[:], in_=vcat[:])
        wself_s = cp.tile([D, D], BF16)
        nc.gpsimd.dma_start(out=wself_s[:], in_=wself[:])
        wcross_s = cp.tile([D, D], F32)
        nc.gpsimd.dma_start(out=wcross_s[:], in_=wcross[:])
        asl_s = cp.tile([P, D], F32)
        nc.gpsimd.dma_start(out=asl_s[:], in_=asl[:])
        adl_s = cp.tile([P, D], F32)
        nc.gpsimd.dma_start(out=adl_s[:], in_=adl[:])
        bw_s = cp.tile([P, R * D], F32)
        nc.gpsimd.dma_start(out=bw_s[:], in_=bw[:])
        bl_s = cp.tile([P, D], F32)
        nc.gpsimd.dma_start(out=bl_s[:], in_=bl[:])
        iota_s = cp.tile([P, P], BF16)
        nc.gpsimd.dma_start(out=iota_s[:], in_=iota[:])
        iden_s = cp.tile([P, P], F32)
        nc.gpsimd.dma_start(out=iden_s[:], in_=iden[:])
        srcg_s = cp.tile([P, TOTC], I32)
        nc.gpsimd.dma_start(out=srcg_s[:], in_=srcg[:])
        argi_s = cp.tile([P, TOTC], I32)
        nc.gpsimd.dma_start(out=argi_s[:], in_=argi[:])
        dstl_s = cp.tile([P, TOTC], BF16)
        nc.gpsimd.dma_start(out=dstl_s[:], in_=dstl[:])

        # ---- Stage A: LN + per-relation features for all nodes ----
        for gt in range(GT):
            xt = sb.tile([P, D], F32, tag="xt")
            nc.gpsimd.dma_start(out=xt[:], in_=x_full[gt * P:(gt + 1) * P, :])
            mu = sb.tile([P, 1], F32, tag="mu")
            nc.vector.tensor_reduce(out=mu[:], in_=xt[:], axis=AX.X, op=ALU.add)
            nc.vector.tensor_scalar_mul(out=mu[:], in0=mu[:], scalar1=1.0 / D)
            xc = sb.tile([P, D], F32, tag="xc")
            nc.vector.tensor_scalar(out=xc[:], in0=xt[:], scalar1=mu[:],
                                    scalar2=None, op0=ALU.subtract)
            sq = sb.tile([P, D], F32, tag="sq")
            nc.scalar.activation(out=sq[:], in_=xc[:], func=AF.Square)
            var = sb.tile([P, 1], F32, tag="var")
            nc.vector.tensor_reduce(out=var[:], in_=sq[:], axis=AX.X, op=ALU.add)
            nc.vector.tensor_scalar(out=var[:], in0=var[:], scalar1=1.0 / D,
                                    scalar2=1e-5, op0=ALU.mult, op1=ALU.add)
            sd = sb.tile([P, 1], F32, tag="sd")
            nc.scalar.activation(out=sd[:], in_=var[:], func=AF.Sqrt)
            rs = sb.tile([P, 1], F32, tag="rs")
            nc.vector.reciprocal(out=rs[:], in_=sd[:])
            xn = sb.tile([P, D], F32, tag="xn")
            nc.vector.tensor_scalar_mul(out=xn[:], in0=xc[:], scalar1=rs[:])
            tp = psA.tile([P, P], F32, tag="tp")
            nc.tensor.transpose(out=tp[:], in_=xn[:], identity=iden_s[:])
            xnT = sb.tile([P, P], BF16, tag="xnT")
            nc.vector.tensor_copy(out=xnT[:], in_=tp[:])
            for r in range(R):
                fm = psA.tile([P, FD], F32, tag="fm")
                nc.tensor.matmul(out=fm[:], lhsT=xnT[:],
                                 rhs=wcat_s[:, r * FD:(r + 1) * FD],
                                 start=True, stop=True)
                fc = sb.tile([P, FD], BF16, tag="fc")
                nc.vector.tensor_copy(out=fc[:], in_=fm[:])
                nc.gpsimd.dma_start(
                    out=feat[r * NPAD + gt * P: r * NPAD + (gt + 1) * P, :],
                    in_=fc[:])
            if gt < T:
                am = psA.tile([P, FD], F32, tag="fm")
                nc.tensor.matmul(out=am[:, :R * H], lhsT=xnT[:], rhs=vcat_s[:],
                                 start=True, stop=True)
                ac = sb.tile([P, R * H], BF16, tag="ac")
                nc.vector.tensor_copy(out=ac[:], in_=am[:, :R * H])
                for r in range(R):
                    nc.gpsimd.dma_start(
                        out=arrel[r * S + gt * P: r * S + (gt + 1) * P, :],
                        in_=ac[:, r * H:(r + 1) * H])
                sm_ = psA.tile([P, FD], F32, tag="fm")
                nc.tensor.matmul(out=sm_[:, :D], lhsT=xnT[:], rhs=wself_s[:],
                                 start=True, stop=True)
                sc = sb.tile([P, D], F32, tag="sc")
                nc.vector.tensor_copy(out=sc[:], in_=sm_[:, :D])
                nc.gpsimd.dma_start(out=sown[gt * P:(gt + 1) * P, :],
                                  in_=sc[:])

        # ---- Stage B: edge aggregation + lang softmax, per owned tile ----
        c = 0
        for t in range(T):
            maskp = lb.tile([P, (R + 1) * H], F32, tag="maskp")
            nc.vector.memset(maskp[:, 0:H], 1.0)
            vts = []
            for r in range(R):
                Kt = K[t][r]
                num_ps = psB.tile([P, D], F32, tag="num")
                den_ps = psB.tile([P, H], F32, tag="den")
                for k in range(Kt):
                    G = eb.tile([P, FD], BF16, tag="G")
                    nc.gpsimd.indirect_dma_start(
                        out=G[:], out_offset=None, in_=feat[:],
                        in_offset=IndirectOffsetOnAxis(ap=srcg_s[:, c:c + 1], axis=0))
                    Aar = eb.tile([P, H], BF16, tag="Aar")
                    nc.gpsimd.indirect_dma_start(
                        out=Aar[:], out_offset=None, in_=arrel[:],
                        in_offset=IndirectOffsetOnAxis(ap=argi_s[:, c:c + 1], axis=0))
                    lg = eb.tile([P, H], F32, tag="lg")
                    nc.vector.tensor_add(out=lg[:], in0=G[:, D:FD], in1=Aar[:])
                    l2 = eb.tile([P, H], F32, tag="l2")
                    nc.vector.tensor_scalar_mul(out=l2[:], in0=lg[:], scalar1=0.2)
                    lr = eb.tile([P, H], F32, tag="lr")
                    nc.vector.tensor_tensor(out=lr[:], in0=lg[:], in1=l2[:],
                                            op=ALU.max)
                    w = eb.tile([P, H], F32, tag="w")
                    nc.scalar.activation(out=w[:], in_=lr[:], func=AF.Exp)
                    wb = eb.tile([P, H], BF16, tag="wb")
                    nc.vector.tensor_copy(out=wb[:], in_=w[:])
                    Sm = eb.tile([P, P], BF16, tag="Sm")
                    nc.vector.tensor_tensor(
                        out=Sm[:], in0=dstl_s[:, c:c + 1].to_broadcast([P, P]),
                        in1=iota_s[:], op=ALU.is_equal)
                    V = eb.tile([P, D], BF16, tag="V")
                    nc.vector.tensor_tensor(
                        out=V[:].rearrange("p (h c) -> p h c", c=C),
                        in0=G[:, 0:D].rearrange("p (h c) -> p h c", c=C),
                        in1=wb[:, :, None].to_broadcast([P, H, C]),
                        op=ALU.mult)
                    nc.tensor.matmul(out=num_ps[:], lhsT=Sm[:], rhs=V[:],
                                     start=(k == 0), stop=(k == Kt - 1))
                    nc.tensor.matmul(out=den_ps[:], lhsT=Sm[:], rhs=wb[:],
                                     start=(k == 0), stop=(k == Kt - 1))
                    c += 1
                den1 = eb.tile([P, H], F32, tag="den1")
                nc.vector.tensor_scalar_max(out=den1[:], in0=den_ps[:],
                                            scalar1=1e-6)
                rec = eb.tile([P, H], F32, tag="rec")
                nc.vector.reciprocal(out=rec[:], in_=den1[:])
                nc.vector.tensor_scalar(
                    out=maskp[:, (r + 1) * H:(r + 2) * H], in0=den_ps[:],
                    scalar1=0.0, scalar2=None, op0=ALU.is_gt)
                O = eb.tile([P, D], F32, tag="O")
                nc.vector.tensor_tensor(
                    out=O[:].rearrange("p (h c) -> p h c", c=C),
                    in0=num_ps[:].rearrange("p (h c) -> p h c", c=C),
                    in1=rec[:, :, None].to_broadcast([P, H, C]),
                    op=ALU.mult)
                nc.vector.tensor_add(out=O[:], in0=O[:],
                                     in1=bw_s[:, r * D:(r + 1) * D])
                g = eb.tile([P, D], F32, tag="g")
                nc.scalar.activation(out=g[:], in_=O[:], func=AF.Gelu)
                tpb = psA.tile([P, P], F32, tag="tp")
                nc.tensor.transpose(out=tpb[:], in_=g[:], identity=iden_s[:])
                gT = eb.tile([P, P], F32, tag="gT")
                nc.vector.tensor_copy(out=gT[:], in_=tpb[:])
                v_ps = psB.tile([P, D], F32, tag="vps")
                nc.tensor.matmul(out=v_ps[:], lhsT=gT[:], rhs=wcross_s[:],
                                 start=True, stop=True)
                vr = lb.tile([P, D], F32, tag=f"v{r + 1}")
                nc.vector.tensor_copy(out=vr[:], in_=v_ps[:])
                vts.append(vr)

            # lang-level GAT over 6 feature rows for this tile
            v0 = lb.tile([P, D], F32, tag="v0")
            nc.gpsimd.dma_start(out=v0[:], in_=sown[t * P:(t + 1) * P, :])
            vall = [v0] + vts
            alp = lb.tile([P, (R + 1) * H], F32, tag="alp")
            tmp = lb.tile([P, D], F32, tag="ltmp")
            for kk in range(R + 1):
                nc.vector.tensor_tensor(out=tmp[:], in0=vall[kk][:],
                                        in1=asl_s[:], op=ALU.mult)
                nc.vector.tensor_reduce(
                    out=alp[:, kk * H:(kk + 1) * H],
                    in_=tmp[:].rearrange("p (h c) -> p h c", c=C),
                    axis=AX.X, op=ALU.add)
            arl = lb.tile([P, H], F32, tag="arl")
            nc.vector.tensor_tensor(out=tmp[:], in0=v0[:], in1=adl_s[:],
                                    op=ALU.mult)
            nc.vector.tensor_reduce(
                out=arl[:], in_=tmp[:].rearrange("p (h c) -> p h c", c=C),
                axis=AX.X, op=ALU.add)
            lgp = lb.tile([P, (R + 1) * H], F32, tag="lgp")
            nc.vector.tensor_tensor(
                out=lgp[:].rearrange("p (k h) -> p k h", h=H),
                in0=alp[:].rearrange("p (k h) -> p k h", h=H),
                in1=arl[:, None, :].to_broadcast([P, R + 1, H]),
                op=ALU.add)
            l2p = lb.tile([P, (R + 1) * H], F32, tag="l2p")
            nc.vector.tensor_scalar_mul(out=l2p[:], in0=lgp[:], scalar1=0.2)
            nc.vector.tensor_tensor(out=lgp[:], in0=lgp[:], in1=l2p[:],
                                    op=ALU.max)
            lm = lb.tile([P, (R + 1) * H], F32, tag="lm")
            nc.vector.tensor_tensor(out=lm[:], in0=lgp[:], in1=maskp[:],
                                    op=ALU.mult)
            mneg = lb.tile([P, (R + 1) * H], F32, tag="mneg")
            nc.vector.tensor_scalar(out=mneg[:], in0=maskp[:], scalar1=1.0,
                                    scalar2=-NEGM, op0=ALU.subtract,
                                    op1=ALU.mult)
            nc.vector.tensor_add(out=lm[:], in0=lm[:], in1=mneg[:])
            ep = lb.tile([P, (R + 1) * H], F32, tag="ep")
            nc.scalar.activation(out=ep[:], in_=lm[:], func=AF.Exp)
            dl = lb.tile([P, H], F32, tag="dl")
            nc.vector.tensor_copy(out=dl[:], in_=ep[:, 0:H])
            for kk in range(1, R + 1):
                nc.vector.tensor_add(out=dl[:], in0=dl[:],
                                     in1=ep[:, kk * H:(kk + 1) * H])
            rl = lb.tile([P, H], F32, tag="rl")
            nc.vector.reciprocal(out=rl[:], in_=dl[:])
            acc = lb.tile([P, D], F32, tag="acc")
            wg = lb.tile([P, H], F32, tag="wg")
            t2 = lb.tile([P, D], F32, tag="t2")
            for kk in range(R + 1):
                nc.vector.tensor_tensor(out=wg[:], in0=ep[:, kk * H:(kk + 1) * H],
                                        in1=rl[:], op=ALU.mult)
                dst_t = acc if kk == 0 else t2
                nc.vector.tensor_tensor(
                    out=dst_t[:].rearrange("p (h c) -> p h c", c=C),
                    in0=vall[kk][:].rearrange("p (h c) -> p h c", c=C),
                    in1=wg[:, :, None].to_broadcast([P, H, C]),
                    op=ALU.mult)
                if kk > 0:
                    nc.vector.tensor_add(out=acc[:], in0=acc[:], in1=t2[:])
            nc.vector.tensor_add(out=acc[:], in0=acc[:], in1=bl_s[:])
            go = lb.tile([P, D], F32, tag="go")
            nc.scalar.activation(out=go[:], in_=acc[:], func=AF.Gelu)
            xr = lb.tile([P, D], F32, tag="xr")
            nc.gpsimd.dma_start(out=xr[:], in_=x_full[t * P:(t + 1) * P, :])
            nc.vector.tensor_add(out=go[:], in0=go[:], in1=xr[:])
            nc.gpsimd.dma_start(out=out[t * P:(t + 1) * P, :], in_=go[:])
    return nc


def _prep(x_inp, edge_index, edge_type, W_self, W_word, att_src_word,
          att_dst_word, bias_word, W_cross, att_src_lang, att_dst_lang,
          bias_lang):
    xpad = np.zeros((NPAD, D), np.float32)
    xpad[:N] = x_inp.astype(np.float32)
    src_all = edge_index[0].astype(np.int64)
    dst_all = edge_index[1].astype(np.int64)
    et_all = edge_type.astype(np.int64)

    # shared params
    Wcat = np.zeros((D, R * FD), np.float32)
    Vcat = np.zeros((D, R * H), np.float32)
    for r in range(R):
        Wr = W_word[r].astype(np.float32)               # [D, D]
        u = np.einsum('dhc,hc->dh', Wr.reshape(D, H, C),
                      att_src_word[r].astype(np.float32))
        v = np.einsum('dhc,hc->dh', Wr.reshape(D, H, C),
                      att_dst_word[r].astype(np.float32))
        Wcat[:, r * FD:r * FD + D] = Wr
        Wcat[:, r * FD + D:(r + 1) * FD] = u
        Vcat[:, r * H:(r + 1) * H] = v
    params = {
        "wcat": Wcat.astype(ml_dtypes.bfloat16),
        "vcat": Vcat.astype(ml_dtypes.bfloat16),
        "wself": W_self.astype(ml_dtypes.bfloat16),
        "wcross": W_cross.astype(np.float32),
        "asl_rep": np.tile(att_src_lang.astype(np.float32).reshape(1, D), (P, 1)),
        "adl_rep": np.tile(att_dst_lang.astype(np.float32).reshape(1, D), (P, 1)),
        "bw_rep": np.tile(bias_word.astype(np.float32).reshape(1, R * D), (P, 1)),
        "bl_rep": np.tile(bias_lang.astype(np.float32).reshape(1, D), (P, 1)),
        "iota_f": np.tile(np.arange(P, dtype=np.float32)[None, :], (P, 1)).astype(ml_dtypes.bfloat16),
        "ident_f": np.eye(P, dtype=np.float32),
    }

    # per-core edge binning
    core_of = dst_all // S
    percore = []
    cnts = np.zeros((M, T, R), np.int64)
    for m in range(M):
        sel = core_of == m
        srcm, dstm, etm = src_all[sel], dst_all[sel], et_all[sel]
        pos = np.empty(NPAD, np.int64)
        pos[m * S:(m + 1) * S] = np.arange(S)
        pos[:m * S] = S + np.arange(m * S)
        pos[(m + 1) * S:] = np.arange((m + 1) * S, NPAD)
        src_l = pos[srcm]
        dst_l = dstm - m * S
        t_loc = dst_l // P
        order = np.lexsort((dst_l % P, etm, t_loc))
        src_l, dst_l, etm, t_loc = (src_l[order], dst_l[order], etm[order],
                                    t_loc[order])
        cnts[m] = np.bincount(t_loc * R + etm, minlength=T * R).reshape(T, R)
        percore.append((pos, src_l, dst_l, etm, t_loc))

    K = np.maximum(1, -(-cnts.max(axis=0) // P))        # [T, R] chunk counts
    TOTC = int(K.sum())
    coff = np.zeros((T, R), np.int64)                    # chunk offsets
    coff.flat[1:] = np.cumsum(K.flat)[:-1]

    in_maps = []
    for m in range(M):
        pos, src_l, dst_l, etm, t_loc = percore[m]
        sg = np.zeros(TOTC * P, np.int32)
        ag = np.zeros(TOTC * P, np.int32)
        dl = np.full(TOTC * P, 200.0, np.float32)
        eoff = np.zeros((T, R), np.int64)
        eoff.flat[1:] = np.cumsum(cnts[m].flat)[:-1]
        for t in range(T):
            for r in range(R):
                n_e = cnts[m, t, r]
                if n_e == 0:
                    continue
                o = eoff[t, r]
                slot = coff[t, r] * P + np.arange(n_e)
                rr = etm[o:o + n_e]
                sg[slot] = rr * NPAD + src_l[o:o + n_e]
                ag[slot] = rr * S + dst_l[o:o + n_e]
                dl[slot] = (dst_l[o:o + n_e] % P).astype(np.float32)
        xperm = np.empty((NPAD, D), np.float32)
        xperm[pos] = xpad
        in_maps.append({
            "x_full": xperm,
            "src_gidx": np.ascontiguousarray(sg.reshape(TOTC, P).T),
            "ar_gidx": np.ascontiguousarray(ag.reshape(TOTC, P).T),
            "dstl_f": np.ascontiguousarray(dl.reshape(TOTC, P).T).astype(ml_dtypes.bfloat16),
            **params,
        })
    return K.tolist(), TOTC, in_maps


def kernel(x_inp, node_type, edge_index, edge_type, W_self, W_word,
           att_src_word, att_dst_word, bias_word, W_cross,
           att_src_lang, att_dst_lang, bias_lang):
    global LAST_RESULTS
    K, TOTC, in_maps = _prep(
        np.asarray(x_inp), np.asarray(edge_index), np.asarray(edge_type),
        np.asarray(W_self), np.asarray(W_word), np.asarray(att_src_word),
        np.asarray(att_dst_word), np.asarray(bias_word), np.asarray(W_cross),
        np.asarray(att_src_lang), np.asarray(att_dst_lang),
        np.asarray(bias_lang))
    nc = _build(K, TOTC)
    _split_multiwaits(nc)
    global LAST_NC, LAST_INMAPS
    LAST_NC, LAST_INMAPS = nc, in_maps
    res = run_bass_kernel_spmd(nc, in_maps, list(range(M)),
                               trace=bool(os.environ.get("BASS_TRACE")))
    LAST_RESULTS = res
    out = np.concatenate([res.results[m]["out"] for m in range(M)], axis=0)
    return out[:N].astype(np.float32)



# revision 6
# speedup vs baseline: 3.8342x; 3.8342x over previous
"""AugGraphConv (per-relation GAT + lang-level softmax) on 8 TRN2 NeuronCores.

v2 — transfer-optimized (the axon tunnel at ~36MB/s dominates wall time):
  - x is SHARDED: core m ships only rows [m*S,(m+1)*S) as f16 (1.6MB vs
    25.7MB replicated f32). Stage A (LN + per-relation features) runs on
    owned rows only; the bf16 feature tables are AllGathered on-device
    (5 x 13.7MB over NeuronLink, Shared output buffers).
  - Edge tables ship as u16 src-node ids + u8 dst-slot ids, upconverted
    on device (i32 gather offsets / bf16 one-hot keys).
  - Attention/bias rows ship as one [1,1024] f32 row, broadcast across
    partitions on device with a K=1 matmul; iota/identity are NEFF consts.
  - Output is f16 gelu(...) WITHOUT the +x residual; the host adds x_inp
    in f32 (halves D2H + donated-zero H2D, and improves accuracy).
Compute structure (per core, dst-sharded graph parallel) is as v1:
  edges binned by (own dst tile, relation) into 128-slot chunks; segment
  softmax without max-subtraction; one-hot scatter-add via PE matmuls.
"""

import os
import numpy as np
import ml_dtypes
from contextlib import ExitStack

import concourse.bass as bass
import concourse.mybir as mybir
from concourse.bass import IndirectOffsetOnAxis
from concourse.tile import TileContext
from concourse.bass_utils import run_bass_kernel_spmd

N, D, H, R, C = 50000, 128, 8, 5, 16
P = 128
M = 8
NPAD = 50176            # 392 * 128, divisible by M*P
S = NPAD // M           # 6272 rows per core
T = S // P              # 49 owned tiles per core
FD = D + H              # 136: [xw | al]
ARPAD = R * S + 2 * P   # arrel rows incl. pad region (covers idx <= 31432)
F32 = mybir.dt.float32
F16 = mybir.dt.float16
BF16 = mybir.dt.bfloat16
I32 = mybir.dt.int32
U16 = mybir.dt.uint16
U8 = mybir.dt.uint8
AF = mybir.ActivationFunctionType
ALU = mybir.AluOpType
AX = mybir.AxisListType
NEGM = -30.0            # softmax mask value (exp(-30) ~ 1e-13, negligible)

LAST_RESULTS = None     # test.py reads exec_time_ns / profile from here


def _split_multiwaits(nc):
    """This toolchain's walrus codegen allows only one sem-wait per
    instruction; hoist extra waits into preceding NoOps on the same engine
    (sequencer executes them in program order, so semantics are identical)."""
    n_split = 0
    for _, bbwrap in nc.bb_map.items():
        bb = bbwrap.bb
        out = []
        changed = False
        for inst in list(bb.instructions):
            si = inst.sync_info
            if si is not None and si.on_wait is not None and len(si.on_wait) > 1:
                waits = list(si.on_wait)
                for w in waits[:-1]:
                    out.append(mybir.InstNoOp(
                        name=nc.get_next_instruction_name(),
                        engine=inst.engine, ins=[], outs=[],
                        sync_info=mybir.SyncInfo(on_wait=[w], on_update=[])))
                    n_split += 1
                si.on_wait = waits[-1:]
                inst.sync_info = si
                changed = True
            out.append(inst)
        if changed:
            bb.instructions = out
    return n_split


def _build(K, TOTC):
    nc = bass.Bass(num_devices=M)
    x_sh = nc.declare_dram_parameter("x_shard", [S, D], F16, isOutput=False)
    srcu = nc.declare_dram_parameter("src_u16", [P, TOTC], U16, isOutput=False)
    dstu = nc.declare_dram_parameter("dst_u8", [P, TOTC], U8, isOutput=False)
    wcat = nc.declare_dram_parameter("wcat", [D, R * FD], BF16, isOutput=False)
    vcat = nc.declare_dram_parameter("vcat", [D, R * H], BF16, isOutput=False)
    wself = nc.declare_dram_parameter("wself", [D, D], BF16, isOutput=False)
    wcross = nc.declare_dram_parameter("wcross", [D, D], BF16, isOutput=False)
    prow = nc.declare_dram_parameter("prow", [1, 8 * D], F32, isOutput=False)
    out = nc.declare_dram_parameter("out", [S, D], F16, isOutput=True)

    ident_d = nc.inline_tensor(np.eye(P, dtype=np.float32), name="ident_c")
    iorow_d = nc.inline_tensor(
        np.tile(np.arange(P, dtype=np.float32)[None, :], (P, 1))
        .astype(ml_dtypes.bfloat16), name="iorow_c")

    featl = [nc.dram_tensor(f"featl{r}", [S, FD], BF16) for r in range(R)]
    featg = [nc.dram_tensor(f"featg{r}", [NPAD, FD], BF16, addr_space="Shared")
             for r in range(R)]
    arrel = nc.dram_tensor("ar_rel", [ARPAD, H], BF16)

    groups = [list(range(M))]

    with TileContext(nc) as tc, ExitStack() as ctx:
        cp = ctx.enter_context(tc.tile_pool(name="const", bufs=1))
        so = ctx.enter_context(tc.tile_pool(name="sown", bufs=1))
        sb = ctx.enter_context(tc.tile_pool(name="sb", bufs=3))
        eb = ctx.enter_context(tc.tile_pool(name="eb", bufs=4))
        lb = ctx.enter_context(tc.tile_pool(name="lb", bufs=2))
        psA = ctx.enter_context(tc.tile_pool(name="psA", bufs=2, space="PSUM"))
        psB = ctx.enter_context(tc.tile_pool(name="psB", bufs=1, space="PSUM"))

        # ---- persistent constants / index arrays ----
        wcat_s = cp.tile([D, R * FD], BF16)
        nc.gpsimd.dma_start(out=wcat_s[:], in_=wcat[:])
        vcat_s = cp.tile([D, R * H], BF16)
        nc.gpsimd.dma_start(out=vcat_s[:], in_=vcat[:])
        wself_s = cp.tile([D, D], BF16)
        nc.gpsimd.dma_start(out=wself_s[:], in_=wself[:])
        wcross_s = cp.tile([D, D], BF16)
        nc.gpsimd.dma_start(out=wcross_s[:], in_=wcross[:])
        iden_s = cp.tile([P, P], F32)
        nc.gpsimd.dma_start(out=iden_s[:], in_=ident_d[:])
        iorow_s = cp.tile([P, P], BF16)
        nc.gpsimd.dma_start(out=iorow_s[:], in_=iorow_d[:])

        srcu_s = cp.tile([P, TOTC], U16)
        nc.gpsimd.dma_start(out=srcu_s[:], in_=srcu[:])
        dstu_s = cp.tile([P, TOTC], U8)
        nc.gpsimd.dma_start(out=dstu_s[:], in_=dstu[:])
        srci_s = cp.tile([P, TOTC], I32)
        nc.vector.tensor_copy(out=srci_s[:], in_=srcu_s[:])
        dsti_s = cp.tile([P, TOTC], I32)
        nc.vector.tensor_copy(out=dsti_s[:], in_=dstu_s[:])
        dstb_s = cp.tile([P, TOTC], BF16)
        nc.vector.tensor_copy(out=dstb_s[:], in_=dstu_s[:])

        # ---- broadcast param row [1,1024] -> [P,1024] via K=1 matmul ----
        prow_s = cp.tile([1, 8 * D], F32)
        nc.gpsimd.dma_start(out=prow_s[:], in_=prow[:])
        ones_s = cp.tile([1, P], F32)
        nc.vector.memset(ones_s[:], 1.0)
        params_s = cp.tile([P, 8 * D], F32)
        for h in range(8):
            pr_ps = psA.tile([P, P], F32, tag="tp")
            nc.tensor.matmul(out=pr_ps[:], lhsT=ones_s[:],
                             rhs=prow_s[:, h * D:(h + 1) * D],
                             start=True, stop=True)
            nc.vector.tensor_copy(out=params_s[:, h * D:(h + 1) * D],
                                  in_=pr_ps[:])
        asl_s = params_s[:, 0:D]
        adl_s = params_s[:, D:2 * D]
        bl_s = params_s[:, 2 * D:3 * D]
        bw_s = params_s[:, 3 * D:8 * D]

        # zero the arrel pad region (gathers may touch rows >= R*S)
        zero_s = cp.tile([P, H], BF16)
        nc.vector.memset(zero_s[:], 0.0)
        for i in range(2):
            nc.gpsimd.dma_start(out=arrel[R * S + i * P:R * S + (i + 1) * P, :],
                                in_=zero_s[:])

        # ---- Stage A: LN + per-relation features for OWNED nodes only ----
        sown_tiles = []
        for t in range(T):
            xt16 = sb.tile([P, D], F16, tag="xt16")
            nc.gpsimd.dma_start(out=xt16[:], in_=x_sh[t * P:(t + 1) * P, :])
            xt = sb.tile([P, D], F32, tag="xt")
            nc.vector.tensor_copy(out=xt[:], in_=xt16[:])
            mu = sb.tile([P, 1], F32, tag="mu")
            nc.vector.tensor_reduce(out=mu[:], in_=xt[:], axis=AX.X, op=ALU.add)
            nc.vector.tensor_scalar_mul(out=mu[:], in0=mu[:], scalar1=1.0 / D)
            xc = sb.tile([P, D], F32, tag="xc")
            nc.vector.tensor_scalar(out=xc[:], in0=xt[:], scalar1=mu[:],
                                    scalar2=None, op0=ALU.subtract)
            sq = sb.tile([P, D], F32, tag="sq")
            nc.scalar.activation(out=sq[:], in_=xc[:], func=AF.Square)
            var = sb.tile([P, 1], F32, tag="var")
            nc.vector.tensor_reduce(out=var[:], in_=sq[:], axis=AX.X, op=ALU.add)
            nc.vector.tensor_scalar(out=var[:], in0=var[:], scalar1=1.0 / D,
                                    scalar2=1e-5, op0=ALU.mult, op1=ALU.add)
            sd = sb.tile([P, 1], F32, tag="sd")
            nc.scalar.activation(out=sd[:], in_=var[:], func=AF.Sqrt)
            rs = sb.tile([P, 1], F32, tag="rs")
            nc.vector.reciprocal(out=rs[:], in_=sd[:])
            xn = sb.tile([P, D], F32, tag="xn")
            nc.vector.tensor_scalar_mul(out=xn[:], in0=xc[:], scalar1=rs[:])
            tp = psA.tile([P, P], F32, tag="tp")
            nc.tensor.transpose(out=tp[:], in_=xn[:], identity=iden_s[:])
            xnT = sb.tile([P, P], BF16, tag="xnT")
            nc.vector.tensor_copy(out=xnT[:], in_=tp[:])
            for r in range(R):
                fm = psA.tile([P, FD], F32, tag="fm")
                nc.tensor.matmul(out=fm[:], lhsT=xnT[:],
                                 rhs=wcat_s[:, r * FD:(r + 1) * FD],
                                 start=True, stop=True)
                fc = sb.tile([P, FD], BF16, tag="fc")
                nc.vector.tensor_copy(out=fc[:], in_=fm[:])
                nc.gpsimd.dma_start(out=featl[r][t * P:(t + 1) * P, :], in_=fc[:])
            am = psA.tile([P, FD], F32, tag="fm")
            nc.tensor.matmul(out=am[:, :R * H], lhsT=xnT[:], rhs=vcat_s[:],
                             start=True, stop=True)
            ac = sb.tile([P, R * H], BF16, tag="ac")
            nc.vector.tensor_copy(out=ac[:], in_=am[:, :R * H])
            for r in range(R):
                nc.gpsimd.dma_start(
                    out=arrel[r * S + t * P:r * S + (t + 1) * P, :],
                    in_=ac[:, r * H:(r + 1) * H])
            sm_ = psA.tile([P, FD], F32, tag="fm")
            nc.tensor.matmul(out=sm_[:, :D], lhsT=xnT[:], rhs=wself_s[:],
                             start=True, stop=True)
            sc = so.tile([P, D], F32, tag=f"sown{t}")
            nc.vector.tensor_copy(out=sc[:], in_=sm_[:, :D])
            sown_tiles.append(sc)

        # ---- AllGather per-relation feature tables across the 8 cores ----
        for r in range(R):
            nc.gpsimd.collective_compute(
                "AllGather", ALU.bypass, replica_groups=groups,
                ins=[featl[r][:]], outs=[featg[r][:]])

        # ---- Stage B: edge aggregation + lang softmax, per owned tile ----
        c = 0
        for t in range(T):
            maskp = lb.tile([P, (R + 1) * H], F32, tag="maskp")
            nc.vector.memset(maskp[:, 0:H], 1.0)
            vts = []
            for r in range(R):
                Kt = K[t][r]
                nd_ps = psB.tile([P, FD], F32, tag="nd")
                num_ps = nd_ps[:, 0:D]
                den_ps = nd_ps[:, D:FD]
                for k in range(Kt):
                    G = eb.tile([P, FD], BF16, tag="G")
                    nc.gpsimd.indirect_dma_start(
                        out=G[:], out_offset=None, in_=featg[r][:],
                        in_offset=IndirectOffsetOnAxis(ap=srci_s[:, c:c + 1], axis=0))
                    ari = eb.tile([P, 1], I32, tag="ari")
                    nc.vector.tensor_scalar(out=ari[:], in0=dsti_s[:, c:c + 1],
                                            scalar1=r * S + t * P, scalar2=None,
                                            op0=ALU.add)
                    Aar = eb.tile([P, H], BF16, tag="Aar")
                    nc.gpsimd.indirect_dma_start(
                        out=Aar[:], out_offset=None, in_=arrel[:],
                        in_offset=IndirectOffsetOnAxis(ap=ari[:], axis=0))
                    lg = eb.tile([P, H], F32, tag="lg")
                    nc.vector.tensor_add(out=lg[:], in0=G[:, D:FD], in1=Aar[:])
                    l2 = eb.tile([P, H], F32, tag="l2")
                    nc.vector.tensor_scalar_mul(out=l2[:], in0=lg[:], scalar1=0.2)
                    lr = eb.tile([P, H], F32, tag="lr")
                    nc.vector.tensor_tensor(out=lr[:], in0=lg[:], in1=l2[:],
                                            op=ALU.max)
                    wb = eb.tile([P, H], BF16, tag="wb")
                    nc.scalar.activation(out=wb[:], in_=lr[:], func=AF.Exp)
                    V = eb.tile([P, FD], BF16, tag="V")
                    nc.vector.tensor_copy(out=V[:, D:FD], in_=wb[:])
                    Sm = eb.tile([P, P], BF16, tag="Sm")
                    nc.vector.tensor_tensor(
                        out=Sm[:], in0=dstb_s[:, c:c + 1].to_broadcast([P, P]),
                        in1=iorow_s[:], op=ALU.is_equal)
                    nc.vector.tensor_tensor(
                        out=V[:, 0:D].rearrange("p (h c) -> p h c", c=C),
                        in0=G[:, 0:D].rearrange("p (h c) -> p h c", c=C),
                        in1=wb[:, :, None].to_broadcast([P, H, C]),
                        op=ALU.mult)
                    nc.tensor.matmul(out=nd_ps[:], lhsT=Sm[:], rhs=V[:],
                                     start=(k == 0), stop=(k == Kt - 1))
                    c += 1
                den1 = eb.tile([P, H], F32, tag="den1")
                nc.vector.tensor_scalar_max(out=den1[:], in0=den_ps[:],
                                            scalar1=1e-6)
                rec = eb.tile([P, H], F32, tag="rec")
                nc.vector.reciprocal(out=rec[:], in_=den1[:])
                nc.vector.tensor_scalar(
                    out=maskp[:, (r + 1) * H:(r + 2) * H], in0=den_ps[:],
                    scalar1=0.0, scalar2=None, op0=ALU.is_gt)
                O = eb.tile([P, D], F32, tag="O")
                nc.vector.tensor_tensor(
                    out=O[:].rearrange("p (h c) -> p h c", c=C),
                    in0=num_ps[:].rearrange("p (h c) -> p h c", c=C),
                    in1=rec[:, :, None].to_broadcast([P, H, C]),
                    op=ALU.mult)
                nc.vector.tensor_add(out=O[:], in0=O[:],
                                     in1=bw_s[:, r * D:(r + 1) * D])
                g = eb.tile([P, D], F32, tag="g")
                nc.scalar.activation(out=g[:], in_=O[:], func=AF.Gelu)
                tpb = psA.tile([P, P], F32, tag="tp")
                nc.tensor.transpose(out=tpb[:], in_=g[:], identity=iden_s[:])
                gT = eb.tile([P, P], BF16, tag="gT")
                nc.vector.tensor_copy(out=gT[:], in_=tpb[:])
                v_ps = psB.tile([P, D], F32, tag="vps")
                nc.tensor.matmul(out=v_ps[:], lhsT=gT[:], rhs=wcross_s[:],
                                 start=True, stop=True)
                vr = lb.tile([P, D], F32, tag=f"v{r + 1}")
                nc.vector.tensor_copy(out=vr[:], in_=v_ps[:])
                vts.append(vr)

            # lang-level GAT over 6 feature rows for this tile
            v0 = sown_tiles[t]
            vall = [v0] + vts
            alp = lb.tile([P, (R + 1) * H], F32, tag="alp")
            tmp = lb.tile([P, D], F32, tag="ltmp")
            for kk in range(R + 1):
                nc.vector.tensor_tensor(out=tmp[:], in0=vall[kk][:],
                                        in1=asl_s, op=ALU.mult)
                nc.vector.tensor_reduce(
                    out=alp[:, kk * H:(kk + 1) * H],
                    in_=tmp[:].rearrange("p (h c) -> p h c", c=C),
                    axis=AX.X, op=ALU.add)
            arl = lb.tile([P, H], F32, tag="arl")
            nc.vector.tensor_tensor(out=tmp[:], in0=v0[:], in1=adl_s,
                                    op=ALU.mult)
            nc.vector.tensor_reduce(
                out=arl[:], in_=tmp[:].rearrange("p (h c) -> p h c", c=C),
                axis=AX.X, op=ALU.add)
            lgp = lb.tile([P, (R + 1) * H], F32, tag="lgp")
            nc.vector.tensor_tensor(
                out=lgp[:].rearrange("p (k h) -> p k h", h=H),
                in0=alp[:].rearrange("p (k h) -> p k h", h=H),
                in1=arl[:, None, :].to_broadcast([P, R + 1, H]),
                op=ALU.add)
            l2p = lb.tile([P, (R + 1) * H], F32, tag="l2p")
            nc.vector.tensor_scalar_mul(out=l2p[:], in0=lgp[:], scalar1=0.2)
            nc.vector.tensor_tensor(out=lgp[:], in0=lgp[:], in1=l2p[:],
                                    op=ALU.max)
            lm = lb.tile([P, (R + 1) * H], F32, tag="lm")
            nc.vector.tensor_tensor(out=lm[:], in0=lgp[:], in1=maskp[:],
                                    op=ALU.mult)
            mneg = lb.tile([P, (R + 1) * H], F32, tag="mneg")
            nc.vector.tensor_scalar(out=mneg[:], in0=maskp[:], scalar1=1.0,
                                    scalar2=-NEGM, op0=ALU.subtract,
                                    op1=ALU.mult)
            nc.vector.tensor_add(out=lm[:], in0=lm[:], in1=mneg[:])
            ep = lb.tile([P, (R + 1) * H], F32, tag="ep")
            nc.scalar.activation(out=ep[:], in_=lm[:], func=AF.Exp)
            dl = lb.tile([P, H], F32, tag="dl")
            nc.vector.tensor_copy(out=dl[:], in_=ep[:, 0:H])
            for kk in range(1, R + 1):
                nc.vector.tensor_add(out=dl[:], in0=dl[:],
                                     in1=ep[:, kk * H:(kk + 1) * H])
            rl = lb.tile([P, H], F32, tag="rl")
            nc.vector.reciprocal(out=rl[:], in_=dl[:])
            acc = lb.tile([P, D], F32, tag="acc")
            wg = lb.tile([P, H], F32, tag="wg")
            t2 = lb.tile([P, D], F32, tag="t2")
            for kk in range(R + 1):
                nc.vector.tensor_tensor(out=wg[:], in0=ep[:, kk * H:(kk + 1) * H],
                                        in1=rl[:], op=ALU.mult)
                dst_t = acc if kk == 0 else t2
                nc.vector.tensor_tensor(
                    out=dst_t[:].rearrange("p (h c) -> p h c", c=C),
                    in0=vall[kk][:].rearrange("p (h c) -> p h c", c=C),
                    in1=wg[:, :, None].to_broadcast([P, H, C]),
                    op=ALU.mult)
                if kk > 0:
                    nc.vector.tensor_add(out=acc[:], in0=acc[:], in1=t2[:])
            nc.vector.tensor_add(out=acc[:], in0=acc[:], in1=bl_s)
            go = lb.tile([P, D], F16, tag="go")
            nc.scalar.activation(out=go[:], in_=acc[:], func=AF.Gelu)
            nc.gpsimd.dma_start(out=out[t * P:(t + 1) * P, :], in_=go[:])
    return nc


def _prep(x_inp, edge_index, edge_type, W_self, W_word, att_src_word,
          att_dst_word, bias_word, W_cross, att_src_lang, att_dst_lang,
          bias_lang):
    xpad = np.zeros((NPAD, D), np.float16)
    xpad[:N] = x_inp.astype(np.float16)
    src_all = edge_index[0].astype(np.int64)
    dst_all = edge_index[1].astype(np.int64)
    et_all = edge_type.astype(np.int64)

    # shared params
    Wcat = np.zeros((D, R * FD), np.float32)
    Vcat = np.zeros((D, R * H), np.float32)
    for r in range(R):
        Wr = W_word[r].astype(np.float32)               # [D, D]
        u = np.einsum('dhc,hc->dh', Wr.reshape(D, H, C),
                      att_src_word[r].astype(np.float32))
        v = np.einsum('dhc,hc->dh', Wr.reshape(D, H, C),
                      att_dst_word[r].astype(np.float32))
        Wcat[:, r * FD:r * FD + D] = Wr
        Wcat[:, r * FD + D:(r + 1) * FD] = u
        Vcat[:, r * H:(r + 1) * H] = v
    prow = np.zeros((1, 8 * D), np.float32)
    prow[0, 0:D] = att_src_lang.astype(np.float32).reshape(D)
    prow[0, D:2 * D] = att_dst_lang.astype(np.float32).reshape(D)
    prow[0, 2 * D:3 * D] = bias_lang.astype(np.float32)
    prow[0, 3 * D:8 * D] = bias_word.astype(np.float32).reshape(R * D)
    params = {
        "wcat": Wcat.astype(ml_dtypes.bfloat16),
        "vcat": Vcat.astype(ml_dtypes.bfloat16),
        "wself": W_self.astype(ml_dtypes.bfloat16),
        "wcross": W_cross.astype(ml_dtypes.bfloat16),
        "prow": prow,
    }

    # per-core edge binning by (dst tile, relation)
    core_of = dst_all // S
    percore = []
    cnts = np.zeros((M, T, R), np.int64)
    for m in range(M):
        sel = core_of == m
        srcm, dstm, etm = src_all[sel], dst_all[sel], et_all[sel]
        dst_l = dstm - m * S
        t_loc = dst_l // P
        order = np.lexsort((etm, t_loc))
        srcm, dst_l, etm, t_loc = (srcm[order], dst_l[order], etm[order],
                                   t_loc[order])
        cnts[m] = np.bincount(t_loc * R + etm, minlength=T * R).reshape(T, R)
        percore.append((srcm, dst_l, etm))

    K = np.maximum(1, -(-cnts.max(axis=0) // P))        # [T, R] chunk counts
    TOTC = int(K.sum())
    coff = np.zeros((T, R), np.int64)                    # chunk offsets
    coff.flat[1:] = np.cumsum(K.flat)[:-1]

    in_maps = []
    for m in range(M):
        srcm, dst_l, etm = percore[m]
        sg = np.zeros(TOTC * P, np.uint16)
        du = np.full(TOTC * P, 200, np.uint8)
        eoff = np.zeros((T, R), np.int64)
        eoff.flat[1:] = np.cumsum(cnts[m].flat)[:-1]
        for t in range(T):
            for r in range(R):
                n_e = cnts[m, t, r]
                if n_e == 0:
                    continue
                o = eoff[t, r]
                slot = coff[t, r] * P + np.arange(n_e)
                sg[slot] = srcm[o:o + n_e]
                du[slot] = (dst_l[o:o + n_e] % P)
        in_maps.append({
            "x_shard": xpad[m * S:(m + 1) * S],
            "src_u16": np.ascontiguousarray(sg.reshape(TOTC, P).T),
            "dst_u8": np.ascontiguousarray(du.reshape(TOTC, P).T),
            **params,
        })
    return K.tolist(), TOTC, in_maps


def kernel(x_inp, node_type, edge_index, edge_type, W_self, W_word,
           att_src_word, att_dst_word, bias_word, W_cross,
           att_src_lang, att_dst_lang, bias_lang):
    global LAST_RESULTS
    x_inp = np.asarray(x_inp)
    K, TOTC, in_maps = _prep(
        x_inp, np.asarray(edge_index), np.asarray(edge_type),
        np.asarray(W_self), np.asarray(W_word), np.asarray(att_src_word),
        np.asarray(att_dst_word), np.asarray(bias_word), np.asarray(W_cross),
        np.asarray(att_src_lang), np.asarray(att_dst_lang),
        np.asarray(bias_lang))
    nc = _build(K, TOTC)
    _split_multiwaits(nc)
    global LAST_NC, LAST_INMAPS
    LAST_NC, LAST_INMAPS = nc, in_maps
    res = run_bass_kernel_spmd(nc, in_maps, list(range(M)),
                               trace=bool(os.environ.get("BASS_TRACE")))
    LAST_RESULTS = res
    gout = np.concatenate([np.asarray(res.results[m]["out"]) for m in range(M)],
                          axis=0)[:N].astype(np.float32)
    return gout + x_inp.astype(np.float32)


# revision 7
# speedup vs baseline: 12.2887x; 3.2050x over previous
"""AugGraphConv (per-relation GAT + lang-level softmax) on 8 TRN2 NeuronCores.

v2 — transfer-optimized (the axon tunnel at ~36MB/s dominates wall time):
  - x is SHARDED: core m ships only rows [m*S,(m+1)*S) as f16 (1.6MB vs
    25.7MB replicated f32). Stage A (LN + per-relation features) runs on
    owned rows only; the bf16 feature tables are AllGathered on-device
    (5 x 13.7MB over NeuronLink, Shared output buffers).
  - Edge tables ship as u16 src-node ids + u8 dst-slot ids, upconverted
    on device (i32 gather offsets / bf16 one-hot keys).
  - Attention/bias rows ship as one [1,1024] f32 row, broadcast across
    partitions on device with a K=1 matmul; iota/identity are NEFF consts.
  - Output is f16 gelu(...) WITHOUT the +x residual; the host adds x_inp
    in f32 (halves D2H + donated-zero H2D, and improves accuracy).
Compute structure (per core, dst-sharded graph parallel) is as v1:
  edges binned by (own dst tile, relation) into 128-slot chunks; segment
  softmax without max-subtraction; one-hot scatter-add via PE matmuls.
"""

import os
import numpy as np
import ml_dtypes
from contextlib import ExitStack

import concourse.bass as bass
import concourse.mybir as mybir
from concourse.bass import IndirectOffsetOnAxis
from concourse.tile import TileContext
from concourse.bass_utils import run_bass_kernel_spmd

N, D, H, R, C = 50000, 128, 8, 5, 16
P = 128
M = 8
NPAD = 50176            # 392 * 128, divisible by M*P
S = NPAD // M           # 6272 rows per core
T = S // P              # 49 owned tiles per core
FD = D + H              # 136: [xw | al]
ARPAD = R * S + 2 * P   # arrel rows incl. pad region (covers idx <= 31432)
F32 = mybir.dt.float32
F16 = mybir.dt.float16
BF16 = mybir.dt.bfloat16
I32 = mybir.dt.int32
U16 = mybir.dt.uint16
U8 = mybir.dt.uint8
AF = mybir.ActivationFunctionType
ALU = mybir.AluOpType
AX = mybir.AxisListType
NEGM = -30.0            # softmax mask value (exp(-30) ~ 1e-13, negligible)

LAST_RESULTS = None     # test.py reads exec_time_ns / profile from here


def _split_multiwaits(nc):
    """This toolchain's walrus codegen allows only one sem-wait per
    instruction; hoist extra waits into preceding NoOps on the same engine
    (sequencer executes them in program order, so semantics are identical)."""
    n_split = 0
    for _, bbwrap in nc.bb_map.items():
        bb = bbwrap.bb
        out = []
        changed = False
        for inst in list(bb.instructions):
            si = inst.sync_info
            if si is not None and si.on_wait is not None and len(si.on_wait) > 1:
                waits = list(si.on_wait)
                for w in waits[:-1]:
                    out.append(mybir.InstNoOp(
                        name=nc.get_next_instruction_name(),
                        engine=inst.engine, ins=[], outs=[],
                        sync_info=mybir.SyncInfo(on_wait=[w], on_update=[])))
                    n_split += 1
                si.on_wait = waits[-1:]
                inst.sync_info = si
                changed = True
            out.append(inst)
        if changed:
            bb.instructions = out
    return n_split


def _build(K, TOTC):
    nc = bass.Bass(num_devices=M)
    x_sh = nc.declare_dram_parameter("x_shard", [S, D], F16, isOutput=False)
    srcu = nc.declare_dram_parameter("src_u16", [P, TOTC], U16, isOutput=False)
    dstu = nc.declare_dram_parameter("dst_u8", [P, TOTC], U8, isOutput=False)
    wcat = nc.declare_dram_parameter("wcat", [D, R * FD], BF16, isOutput=False)
    vcat = nc.declare_dram_parameter("vcat", [D, R * H], BF16, isOutput=False)
    wself = nc.declare_dram_parameter("wself", [D, D], BF16, isOutput=False)
    wcross = nc.declare_dram_parameter("wcross", [D, D], BF16, isOutput=False)
    prow = nc.declare_dram_parameter("prow", [1, 8 * D], F32, isOutput=False)
    out = nc.declare_dram_parameter("out", [S, D], F16, isOutput=True)

    ident_d = nc.inline_tensor(np.eye(P, dtype=np.float32), name="ident_c")
    iorow_d = nc.inline_tensor(
        np.tile(np.arange(P, dtype=np.float32)[None, :], (P, 1))
        .astype(ml_dtypes.bfloat16), name="iorow_c")

    featl = [nc.dram_tensor(f"featl{r}", [S, FD], BF16) for r in range(R)]
    featg = [nc.dram_tensor(f"featg{r}", [NPAD, FD], BF16, addr_space="Shared")
             for r in range(R)]
    arrel = nc.dram_tensor("ar_rel", [ARPAD, H], BF16)

    groups = [list(range(M))]

    with TileContext(nc) as tc, ExitStack() as ctx:
        cp = ctx.enter_context(tc.tile_pool(name="const", bufs=1))
        so = ctx.enter_context(tc.tile_pool(name="sown", bufs=1))
        sb = ctx.enter_context(tc.tile_pool(name="sb", bufs=3))
        eb = ctx.enter_context(tc.tile_pool(name="eb", bufs=4))
        lb = ctx.enter_context(tc.tile_pool(name="lb", bufs=2))
        psA = ctx.enter_context(tc.tile_pool(name="psA", bufs=2, space="PSUM"))
        psB = ctx.enter_context(tc.tile_pool(name="psB", bufs=1, space="PSUM"))

        # ---- persistent constants / index arrays ----
        wcat_s = cp.tile([D, R * FD], BF16)
        nc.gpsimd.dma_start(out=wcat_s[:], in_=wcat[:])
        vcat_s = cp.tile([D, R * H], BF16)
        nc.gpsimd.dma_start(out=vcat_s[:], in_=vcat[:])
        wself_s = cp.tile([D, D], BF16)
        nc.gpsimd.dma_start(out=wself_s[:], in_=wself[:])
        wcross_s = cp.tile([D, D], BF16)
        nc.gpsimd.dma_start(out=wcross_s[:], in_=wcross[:])
        iden_s = cp.tile([P, P], F32)
        nc.gpsimd.dma_start(out=iden_s[:], in_=ident_d[:])
        iorow_s = cp.tile([P, P], BF16)
        nc.gpsimd.dma_start(out=iorow_s[:], in_=iorow_d[:])

        srcu_s = cp.tile([P, TOTC], U16)
        nc.gpsimd.dma_start(out=srcu_s[:], in_=srcu[:])
        dstu_s = cp.tile([P, TOTC], U8)
        nc.gpsimd.dma_start(out=dstu_s[:], in_=dstu[:])
        srci_s = cp.tile([P, TOTC], I32)
        nc.vector.tensor_copy(out=srci_s[:], in_=srcu_s[:])
        dsti_s = cp.tile([P, TOTC], I32)
        nc.vector.tensor_copy(out=dsti_s[:], in_=dstu_s[:])
        dstb_s = cp.tile([P, TOTC], BF16)
        nc.vector.tensor_copy(out=dstb_s[:], in_=dstu_s[:])

        # ---- broadcast param row [1,1024] -> [P,1024] via K=1 matmul ----
        prow_s = cp.tile([1, 8 * D], F32)
        nc.gpsimd.dma_start(out=prow_s[:], in_=prow[:])
        ones_s = cp.tile([1, P], F32)
        nc.vector.memset(ones_s[:], 1.0)
        params_s = cp.tile([P, 8 * D], F32)
        for h in range(8):
            pr_ps = psA.tile([P, P], F32, tag="tp")
            nc.tensor.matmul(out=pr_ps[:], lhsT=ones_s[:],
                             rhs=prow_s[:, h * D:(h + 1) * D],
                             start=True, stop=True)
            nc.vector.tensor_copy(out=params_s[:, h * D:(h + 1) * D],
                                  in_=pr_ps[:])
        asl_s = params_s[:, 0:D]
        adl_s = params_s[:, D:2 * D]
        bl_s = params_s[:, 2 * D:3 * D]
        bw_s = params_s[:, 3 * D:8 * D]

        # zero the arrel pad region (gathers may touch rows >= R*S)
        zero_s = cp.tile([P, H], BF16)
        nc.vector.memset(zero_s[:], 0.0)
        for i in range(2):
            nc.gpsimd.dma_start(out=arrel[R * S + i * P:R * S + (i + 1) * P, :],
                                in_=zero_s[:])

        # ---- Stage A: LN + per-relation features for OWNED nodes only ----
        sown_tiles = []
        for t in range(T):
            xt16 = sb.tile([P, D], F16, tag="xt16")
            nc.gpsimd.dma_start(out=xt16[:], in_=x_sh[t * P:(t + 1) * P, :])
            xt = sb.tile([P, D], F32, tag="xt")
            nc.vector.tensor_copy(out=xt[:], in_=xt16[:])
            mu = sb.tile([P, 1], F32, tag="mu")
            nc.vector.tensor_reduce(out=mu[:], in_=xt[:], axis=AX.X, op=ALU.add)
            nc.vector.tensor_scalar_mul(out=mu[:], in0=mu[:], scalar1=1.0 / D)
            xc = sb.tile([P, D], F32, tag="xc")
            nc.vector.tensor_scalar(out=xc[:], in0=xt[:], scalar1=mu[:],
                                    scalar2=None, op0=ALU.subtract)
            sq = sb.tile([P, D], F32, tag="sq")
            nc.scalar.activation(out=sq[:], in_=xc[:], func=AF.Square)
            var = sb.tile([P, 1], F32, tag="var")
            nc.vector.tensor_reduce(out=var[:], in_=sq[:], axis=AX.X, op=ALU.add)
            nc.vector.tensor_scalar(out=var[:], in0=var[:], scalar1=1.0 / D,
                                    scalar2=1e-5, op0=ALU.mult, op1=ALU.add)
            sd = sb.tile([P, 1], F32, tag="sd")
            nc.scalar.activation(out=sd[:], in_=var[:], func=AF.Sqrt)
            rs = sb.tile([P, 1], F32, tag="rs")
            nc.vector.reciprocal(out=rs[:], in_=sd[:])
            xn = sb.tile([P, D], F32, tag="xn")
            nc.vector.tensor_scalar_mul(out=xn[:], in0=xc[:], scalar1=rs[:])
            tp = psA.tile([P, P], F32, tag="tp")
            nc.tensor.transpose(out=tp[:], in_=xn[:], identity=iden_s[:])
            xnT = sb.tile([P, P], BF16, tag="xnT")
            nc.vector.tensor_copy(out=xnT[:], in_=tp[:])
            for r in range(R):
                fm = psA.tile([P, FD], F32, tag="fm")
                nc.tensor.matmul(out=fm[:], lhsT=xnT[:],
                                 rhs=wcat_s[:, r * FD:(r + 1) * FD],
                                 start=True, stop=True)
                fc = sb.tile([P, FD], BF16, tag="fc")
                nc.vector.tensor_copy(out=fc[:], in_=fm[:])
                nc.gpsimd.dma_start(out=featl[r][t * P:(t + 1) * P, :], in_=fc[:])
            am = psA.tile([P, FD], F32, tag="fm")
            nc.tensor.matmul(out=am[:, :R * H], lhsT=xnT[:], rhs=vcat_s[:],
                             start=True, stop=True)
            ac = sb.tile([P, R * H], BF16, tag="ac")
            nc.vector.tensor_copy(out=ac[:], in_=am[:, :R * H])
            for r in range(R):
                nc.gpsimd.dma_start(
                    out=arrel[r * S + t * P:r * S + (t + 1) * P, :],
                    in_=ac[:, r * H:(r + 1) * H])
            sm_ = psA.tile([P, FD], F32, tag="fm")
            nc.tensor.matmul(out=sm_[:, :D], lhsT=xnT[:], rhs=wself_s[:],
                             start=True, stop=True)
            sc = so.tile([P, D], F32, tag=f"sown{t}")
            nc.vector.tensor_copy(out=sc[:], in_=sm_[:, :D])
            sown_tiles.append(sc)

        # ---- AllGather per-relation feature tables across the 8 cores ----
        for r in range(R):
            nc.gpsimd.collective_compute(
                "AllGather", ALU.bypass, replica_groups=groups,
                ins=[featl[r][:]], outs=[featg[r][:]])

        # ---- Stage B: edge aggregation + lang softmax, per owned tile ----
        c = 0
        for t in range(T):
            maskp = lb.tile([P, (R + 1) * H], F32, tag="maskp")
            nc.vector.memset(maskp[:, 0:H], 1.0)
            vts = []
            for r in range(R):
                Kt = K[t][r]
                nd_ps = psB.tile([P, FD], F32, tag="nd")
                num_ps = nd_ps[:, 0:D]
                den_ps = nd_ps[:, D:FD]
                for k in range(Kt):
                    G = eb.tile([P, FD], BF16, tag="G")
                    nc.gpsimd.indirect_dma_start(
                        out=G[:], out_offset=None, in_=featg[r][:],
                        in_offset=IndirectOffsetOnAxis(ap=srci_s[:, c:c + 1], axis=0))
                    ari = eb.tile([P, 1], I32, tag="ari")
                    nc.vector.tensor_scalar(out=ari[:], in0=dsti_s[:, c:c + 1],
                                            scalar1=r * S + t * P, scalar2=None,
                                            op0=ALU.add)
                    Aar = eb.tile([P, H], BF16, tag="Aar")
                    nc.gpsimd.indirect_dma_start(
                        out=Aar[:], out_offset=None, in_=arrel[:],
                        in_offset=IndirectOffsetOnAxis(ap=ari[:], axis=0))
                    lg = eb.tile([P, H], F32, tag="lg")
                    nc.vector.tensor_add(out=lg[:], in0=G[:, D:FD], in1=Aar[:])
                    l2 = eb.tile([P, H], F32, tag="l2")
                    nc.vector.tensor_scalar_mul(out=l2[:], in0=lg[:], scalar1=0.2)
                    lr = eb.tile([P, H], F32, tag="lr")
                    nc.vector.tensor_tensor(out=lr[:], in0=lg[:], in1=l2[:],
                                            op=ALU.max)
                    wb = eb.tile([P, H], BF16, tag="wb")
                    nc.scalar.activation(out=wb[:], in_=lr[:], func=AF.Exp)
                    V = eb.tile([P, FD], BF16, tag="V")
                    nc.vector.tensor_copy(out=V[:, D:FD], in_=wb[:])
                    Sm = eb.tile([P, P], BF16, tag="Sm")
                    nc.vector.tensor_tensor(
                        out=Sm[:], in0=dstb_s[:, c:c + 1].to_broadcast([P, P]),
                        in1=iorow_s[:], op=ALU.is_equal)
                    nc.vector.tensor_tensor(
                        out=V[:, 0:D].rearrange("p (h c) -> p h c", c=C),
                        in0=G[:, 0:D].rearrange("p (h c) -> p h c", c=C),
                        in1=wb[:, :, None].to_broadcast([P, H, C]),
                        op=ALU.mult)
                    nc.tensor.matmul(out=nd_ps[:], lhsT=Sm[:], rhs=V[:],
                                     start=(k == 0), stop=(k == Kt - 1))
                    c += 1
                den1 = eb.tile([P, H], F32, tag="den1")
                nc.vector.tensor_scalar_max(out=den1[:], in0=den_ps[:],
                                            scalar1=1e-6)
                rec = eb.tile([P, H], F32, tag="rec")
                nc.vector.reciprocal(out=rec[:], in_=den1[:])
                nc.vector.tensor_scalar(
                    out=maskp[:, (r + 1) * H:(r + 2) * H], in0=den_ps[:],
                    scalar1=0.0, scalar2=None, op0=ALU.is_gt)
                O = eb.tile([P, D], F32, tag="O")
                nc.vector.tensor_tensor(
                    out=O[:].rearrange("p (h c) -> p h c", c=C),
                    in0=num_ps[:].rearrange("p (h c) -> p h c", c=C),
                    in1=rec[:, :, None].to_broadcast([P, H, C]),
                    op=ALU.mult)
                nc.vector.tensor_add(out=O[:], in0=O[:],
                                     in1=bw_s[:, r * D:(r + 1) * D])
                g = eb.tile([P, D], F32, tag="g")
                nc.scalar.activation(out=g[:], in_=O[:], func=AF.Gelu)
                tpb = psA.tile([P, P], F32, tag="tp")
                nc.tensor.transpose(out=tpb[:], in_=g[:], identity=iden_s[:])
                gT = eb.tile([P, P], BF16, tag="gT")
                nc.vector.tensor_copy(out=gT[:], in_=tpb[:])
                v_ps = psB.tile([P, D], F32, tag="vps")
                nc.tensor.matmul(out=v_ps[:], lhsT=gT[:], rhs=wcross_s[:],
                                 start=True, stop=True)
                vr = lb.tile([P, D], F32, tag=f"v{r + 1}")
                nc.vector.tensor_copy(out=vr[:], in_=v_ps[:])
                vts.append(vr)

            # lang-level GAT over 6 feature rows for this tile
            v0 = sown_tiles[t]
            vall = [v0] + vts
            alp = lb.tile([P, (R + 1) * H], F32, tag="alp")
            tmp = lb.tile([P, D], F32, tag="ltmp")
            for kk in range(R + 1):
                nc.vector.tensor_tensor(out=tmp[:], in0=vall[kk][:],
                                        in1=asl_s, op=ALU.mult)
                nc.vector.tensor_reduce(
                    out=alp[:, kk * H:(kk + 1) * H],
                    in_=tmp[:].rearrange("p (h c) -> p h c", c=C),
                    axis=AX.X, op=ALU.add)
            arl = lb.tile([P, H], F32, tag="arl")
            nc.vector.tensor_tensor(out=tmp[:], in0=v0[:], in1=adl_s,
                                    op=ALU.mult)
            nc.vector.tensor_reduce(
                out=arl[:], in_=tmp[:].rearrange("p (h c) -> p h c", c=C),
                axis=AX.X, op=ALU.add)
            lgp = lb.tile([P, (R + 1) * H], F32, tag="lgp")
            nc.vector.tensor_tensor(
                out=lgp[:].rearrange("p (k h) -> p k h", h=H),
                in0=alp[:].rearrange("p (k h) -> p k h", h=H),
                in1=arl[:, None, :].to_broadcast([P, R + 1, H]),
                op=ALU.add)
            l2p = lb.tile([P, (R + 1) * H], F32, tag="l2p")
            nc.vector.tensor_scalar_mul(out=l2p[:], in0=lgp[:], scalar1=0.2)
            nc.vector.tensor_tensor(out=lgp[:], in0=lgp[:], in1=l2p[:],
                                    op=ALU.max)
            lm = lb.tile([P, (R + 1) * H], F32, tag="lm")
            nc.vector.tensor_tensor(out=lm[:], in0=lgp[:], in1=maskp[:],
                                    op=ALU.mult)
            mneg = lb.tile([P, (R + 1) * H], F32, tag="mneg")
            nc.vector.tensor_scalar(out=mneg[:], in0=maskp[:], scalar1=1.0,
                                    scalar2=-NEGM, op0=ALU.subtract,
                                    op1=ALU.mult)
            nc.vector.tensor_add(out=lm[:], in0=lm[:], in1=mneg[:])
            ep = lb.tile([P, (R + 1) * H], F32, tag="ep")
            nc.scalar.activation(out=ep[:], in_=lm[:], func=AF.Exp)
            dl = lb.tile([P, H], F32, tag="dl")
            nc.vector.tensor_copy(out=dl[:], in_=ep[:, 0:H])
            for kk in range(1, R + 1):
                nc.vector.tensor_add(out=dl[:], in0=dl[:],
                                     in1=ep[:, kk * H:(kk + 1) * H])
            rl = lb.tile([P, H], F32, tag="rl")
            nc.vector.reciprocal(out=rl[:], in_=dl[:])
            acc = lb.tile([P, D], F32, tag="acc")
            wg = lb.tile([P, H], F32, tag="wg")
            t2 = lb.tile([P, D], F32, tag="t2")
            for kk in range(R + 1):
                nc.vector.tensor_tensor(out=wg[:], in0=ep[:, kk * H:(kk + 1) * H],
                                        in1=rl[:], op=ALU.mult)
                dst_t = acc if kk == 0 else t2
                nc.vector.tensor_tensor(
                    out=dst_t[:].rearrange("p (h c) -> p h c", c=C),
                    in0=vall[kk][:].rearrange("p (h c) -> p h c", c=C),
                    in1=wg[:, :, None].to_broadcast([P, H, C]),
                    op=ALU.mult)
                if kk > 0:
                    nc.vector.tensor_add(out=acc[:], in0=acc[:], in1=t2[:])
            nc.vector.tensor_add(out=acc[:], in0=acc[:], in1=bl_s)
            go = lb.tile([P, D], F16, tag="go")
            nc.scalar.activation(out=go[:], in_=acc[:], func=AF.Gelu)
            nc.gpsimd.dma_start(out=out[t * P:(t + 1) * P, :], in_=go[:])
    return nc


def _prep(x_inp, edge_index, edge_type, W_self, W_word, att_src_word,
          att_dst_word, bias_word, W_cross, att_src_lang, att_dst_lang,
          bias_lang):
    xpad = np.zeros((NPAD, D), np.float16)
    xpad[:N] = x_inp.astype(np.float16)
    src_all = edge_index[0].astype(np.int64)
    dst_all = edge_index[1].astype(np.int64)
    et_all = edge_type.astype(np.int64)

    # shared params
    Wcat = np.zeros((D, R * FD), np.float32)
    Vcat = np.zeros((D, R * H), np.float32)
    for r in range(R):
        Wr = W_word[r].astype(np.float32)               # [D, D]
        u = np.einsum('dhc,hc->dh', Wr.reshape(D, H, C),
                      att_src_word[r].astype(np.float32))
        v = np.einsum('dhc,hc->dh', Wr.reshape(D, H, C),
                      att_dst_word[r].astype(np.float32))
        Wcat[:, r * FD:r * FD + D] = Wr
        Wcat[:, r * FD + D:(r + 1) * FD] = u
        Vcat[:, r * H:(r + 1) * H] = v
    prow = np.zeros((1, 8 * D), np.float32)
    prow[0, 0:D] = att_src_lang.astype(np.float32).reshape(D)
    prow[0, D:2 * D] = att_dst_lang.astype(np.float32).reshape(D)
    prow[0, 2 * D:3 * D] = bias_lang.astype(np.float32)
    prow[0, 3 * D:8 * D] = bias_word.astype(np.float32).reshape(R * D)
    params = {
        "wcat": Wcat.astype(ml_dtypes.bfloat16),
        "vcat": Vcat.astype(ml_dtypes.bfloat16),
        "wself": W_self.astype(ml_dtypes.bfloat16),
        "wcross": W_cross.astype(ml_dtypes.bfloat16),
        "prow": prow,
    }

    # per-core edge binning by (dst tile, relation)
    core_of = dst_all // S
    percore = []
    cnts = np.zeros((M, T, R), np.int64)
    for m in range(M):
        sel = core_of == m
        srcm, dstm, etm = src_all[sel], dst_all[sel], et_all[sel]
        dst_l = dstm - m * S
        t_loc = dst_l // P
        order = np.lexsort((etm, t_loc))
        srcm, dst_l, etm, t_loc = (srcm[order], dst_l[order], etm[order],
                                   t_loc[order])
        cnts[m] = np.bincount(t_loc * R + etm, minlength=T * R).reshape(T, R)
        percore.append((srcm, dst_l, etm))

    K = np.maximum(1, -(-cnts.max(axis=0) // P))        # [T, R] chunk counts
    TOTC = int(K.sum())
    coff = np.zeros((T, R), np.int64)                    # chunk offsets
    coff.flat[1:] = np.cumsum(K.flat)[:-1]

    in_maps = []
    for m in range(M):
        srcm, dst_l, etm = percore[m]
        sg = np.zeros(TOTC * P, np.uint16)
        du = np.full(TOTC * P, 200, np.uint8)
        eoff = np.zeros((T, R), np.int64)
        eoff.flat[1:] = np.cumsum(cnts[m].flat)[:-1]
        for t in range(T):
            for r in range(R):
                n_e = cnts[m, t, r]
                if n_e == 0:
                    continue
                o = eoff[t, r]
                slot = coff[t, r] * P + np.arange(n_e)
                sg[slot] = srcm[o:o + n_e]
                du[slot] = (dst_l[o:o + n_e] % P)
        in_maps.append({
            "x_shard": xpad[m * S:(m + 1) * S],
            "src_u16": np.ascontiguousarray(sg.reshape(TOTC, P).T),
            "dst_u8": np.ascontiguousarray(du.reshape(TOTC, P).T),
            **params,
        })
    return K.tolist(), TOTC, in_maps


class _CachedExec:
    """Compile the bass program once per program signature and keep the
    jitted SPMD callable; repeat executions then only pay H2D + exec + D2H
    (the intended 'steady-state, compile cached' semantics) instead of
    re-tracing/lowering the ~16k-instruction BIR on every call."""

    def __init__(self, nc):
        import jax
        from jax.sharding import Mesh, PartitionSpec, NamedSharding
        from jax.experimental.shard_map import shard_map
        from concourse import bass2jax
        from concourse.bass2jax import _bass_exec_p, install_neuronx_cc_hook

        install_neuronx_cc_hook()
        self.nc = nc
        in_names, out_names, out_avals, zero_templates = [], [], [], []
        pid = nc.partition_id_tensor.name if nc.partition_id_tensor else None
        for alloc in nc.m.functions[0].allocations:
            if not isinstance(alloc, mybir.MemoryLocationSet):
                continue
            name = alloc.memorylocations[0].name
            if alloc.kind == "ExternalInput":
                if name != pid:
                    in_names.append(name)
            elif alloc.kind == "ExternalOutput":
                out_names.append(name)
                shape = tuple(alloc.tensor_shape)
                dtype = mybir.dt.np(alloc.dtype)
                out_avals.append(jax.core.ShapedArray(shape, dtype))
                zero_templates.append((shape, dtype))
        self.n_params = len(in_names)
        self.in_names = in_names + out_names
        self.out_names = out_names
        if pid is not None:
            self.in_names.append(pid)

        def _body(*args):
            operands = list(args)
            if pid is not None:
                operands.append(bass2jax.partition_id_tensor())
            outs = _bass_exec_p.bind(
                *operands, out_avals=tuple(out_avals),
                in_names=tuple(self.in_names), out_names=tuple(out_names),
                lowering_input_output_aliases=(),
                sim_require_finite=True, sim_require_nnan=True, nc=nc)
            return tuple(outs)

        devices = jax.devices()[:M]
        mesh = Mesh(np.asarray(devices), ("core",))
        n_outs = len(out_names)
        self.sharded = jax.jit(
            shard_map(_body, mesh=mesh,
                      in_specs=(PartitionSpec("core"),) * (self.n_params + n_outs),
                      out_specs=(PartitionSpec("core"),) * n_outs,
                      check_rep=False),
            donate_argnums=tuple(range(self.n_params, self.n_params + n_outs)),
            keep_unused=True)
        # donated output buffers are created ON DEVICE (zeros shipped over
        # the host link every call would be pure transfer waste)
        sh = NamedSharding(mesh, PartitionSpec("core"))
        import jax.numpy as jnp
        self.make_zeros = jax.jit(
            lambda: tuple(jnp.zeros((M * s[0], *s[1:]), d)
                          for s, d in zero_templates),
            out_shardings=tuple(sh for _ in zero_templates))

    def run(self, in_maps):
        concat_in = [
            np.concatenate([np.asarray(in_maps[c][name]) for c in range(M)],
                           axis=0)
            for name in self.in_names[:self.n_params]]
        zeros = self.make_zeros()
        out_arrs = self.sharded(*concat_in, *zeros)
        outs = [np.asarray(o) for o in out_arrs]
        return [
            {name: outs[i].reshape(M, -1, *outs[i].shape[1:])[c]
             for i, name in enumerate(self.out_names)}
            for c in range(M)]


_EXEC_CACHE = {}


def _get_exec(K, TOTC):
    key = (tuple(map(tuple, K)), TOTC)
    if key not in _EXEC_CACHE:
        nc = _build(K, TOTC)
        _split_multiwaits(nc)
        _EXEC_CACHE[key] = _CachedExec(nc)
    return _EXEC_CACHE[key]


def rerun():
    """Re-execute the last-compiled program with the last inputs (full
    H2D + device exec + D2H round trip). Used by test.py for steady-state
    timing."""
    return LAST_EXEC.run(LAST_INMAPS)


def kernel(x_inp, node_type, edge_index, edge_type, W_self, W_word,
           att_src_word, att_dst_word, bias_word, W_cross,
           att_src_lang, att_dst_lang, bias_lang):
    global LAST_RESULTS, LAST_NC, LAST_INMAPS, LAST_EXEC
    x_inp = np.asarray(x_inp)
    K, TOTC, in_maps = _prep(
        x_inp, np.asarray(edge_index), np.asarray(edge_type),
        np.asarray(W_self), np.asarray(W_word), np.asarray(att_src_word),
        np.asarray(att_dst_word), np.asarray(bias_word), np.asarray(W_cross),
        np.asarray(att_src_lang), np.asarray(att_dst_lang),
        np.asarray(bias_lang))
    ex = _get_exec(K, TOTC)
    LAST_NC, LAST_INMAPS, LAST_EXEC = ex.nc, in_maps, ex
    results = ex.run(in_maps)
    LAST_RESULTS = None
    gout = np.concatenate([results[m]["out"] for m in range(M)],
                          axis=0)[:N].astype(np.float32)
    return gout + x_inp.astype(np.float32)
